# revision 18
# baseline (speedup 1.0000x reference)
"""LIF spike-train kernel for Trainium2 (Bass/Tile), data-parallel over 8 cores.

Reference semantics (T=4, tau=0.5, thresh=1.0), per element:
    mem = 0
    for t in range(4):
        mem = mem*0.5 + x[t]
        s[t] = (mem - 1 >= 0)
        mem = mem - s[t]

x: [T*B, C, H, W] = [256, 128, 32, 32] f32, viewed as [4, 64, 128, 1024].
Batch dim (64) is sharded 8-ways; each core streams [4, 8, 128, 1024],
flattened to x,y: [T, 128, F] (F = 8192).

Scheme "SignFlow" (measured-engine-balance redesign; ~76us vs the 107.5us
all-DVE baseline):
  - Act (scalar) engine does all compares: t = Sign(u - c) -> int8 {-1,0,1},
    where c = nextafter(1,0). Exact: u >= 1  <=>  u - c > 0, and u == c
    (sign 0) is correctly "no spike". The int8 tile IS the output; the host
    maps byte==1 -> 1.0f. 1-byte output also cuts write traffic 4x.
    Act measured ~2000ns per [128,2048] op, ~32us total.
  - DVE does only the two 2-source ops per step (measured flat ~2290ns per
    [128,2048] op regardless of operand dtypes/spaces; 1-source ops 1216ns):
      reset:     nv_t = max(t_t, 0) - u_t            (= -(u - s), sign
                 absorbed into the next integrate)
      integrate: u_{t+1} = nv_t * -0.5 + x_{t+1}     (= 0.5*v + x)
    fp32 mult by 0.5 is exact, adds round identically to the reference.
    24 ops x 2290 = 54.8us busy, gapless = the critical engine.
  - The reset is emitted one chunk behind its sign (software-pipeline skew)
    so the DVE stream never stalls waiting for Act.
  - Pool/PE idle: gpsimd measured 5.4us (tt) to 37us (ts) per op and its
    SBUF-port contention slows the DVE (tested 90us); a PE/PSUM hybrid
    (Act copy u->psum + PE accum -0.5*I@s_fp8, validated exact on HW) nets
    only ~2us because PSUM fits half the working set and the fp8 compare
    costs 1472 vs 1218 on DVE -- not worth the complexity.
  - Remaining time = ~6.5us engine-wake preamble (bass const-AP memsets on
    slow-waking gpsimd gate the init barrier) + ~7us first-load ramp +
    54.8us DVE + ~5us tail (store + drain + semaphore-clear storm).

All spike decisions are bit-exact vs the fp32 reference (rel err 0.0).
"""

import os
import sys

sys.path.insert(0, "/opt/trn_rl_repo")

import numpy as np

T = 4
B = 64
C = 128
HW = 1024
NCORES = 8
BLOC = B // NCORES  # 8 batch elements per core
F = BLOC * C * HW // 128  # 8192 flat free width per t-block
C_THRESH = float(np.nextafter(np.float32(1.0), np.float32(0.0)))

LAST_EXEC_NS = None
LAST_TRACE = None

_CACHE = {}


def _build_sign():
    """SignFlow kernel: Act compares, DVE 2-src ops, int8 spike output."""
    import concourse.bacc as bacc
    import concourse.mybir as mybir
    from concourse import tile

    f32 = mybir.dt.float32
    i8 = mybir.dt.int8
    A = mybir.AluOpType
    AF = mybir.ActivationFunctionType

    W = min(int(os.environ.get("LIF_W", "2048")), F)  # load-chunk width
    CW = min(int(os.environ.get("LIF_CW", str(W))), W)  # compute-chunk width
    NCH = F // W
    SUB = W // CW
    NCC = F // CW
    assert F % W == 0 and W % CW == 0

    nc = bacc.Bacc("TRN2", target_bir_lowering=False, debug=False, num_devices=NCORES)
    x = nc.dram_tensor("x", [T, 128, F], f32, kind="ExternalInput").ap()
    y = nc.dram_tensor("y", [T, 128, F], i8, kind="ExternalOutput").ap()

    xbufs = int(os.environ.get("LIF_XBUFS", "6"))
    ubufs = int(os.environ.get("LIF_UBUFS", "6"))
    tbufs = int(os.environ.get("LIF_TBUFS", "2"))
    ringsplit = os.environ.get("LIF_RINGSPLIT") == "1"
    st_eng_name = os.environ.get("LIF_STORE_ENG", "sync")
    skew = os.environ.get("LIF_SKEW", "1") == "1"
    chunkstore_all = os.environ.get("LIF_CHUNKSTORE_ALL", "0") == "1"

    with tile.TileContext(nc) as tc:
        with tc.tile_pool(name="p", bufs=xbufs) as pool:
            bias = pool.tile([128, 1], f32, tag="bias", bufs=1)
            warm = pool.tile([128, 1], i8, tag="warm", bufs=1)
            # memset on DVE (not gpsimd): gpsimd's slow wake already gates the
            # init barrier via the bass const-AP memsets; don't add to it.
            nc.vector.memset(bias, -C_THRESH)
            # dummy Sign to pull the ACT table load into the preamble
            nc.scalar.activation(warm, bias, AF.Sign, bias=bias)
            st_eng = nc.scalar if st_eng_name == "scalar" else nc.sync

            # t=0 runs at fine granularity (W0) so the first compute starts
            # as soon as a small first load lands; resets write into
            # CW-wide nv tiles so t=1.. runs at coarse granularity.
            W0 = min(int(os.environ.get("LIF_W0", str(CW))), CW)
            nvs = {}
            for t in range(T):
                # loads (load-chunk granularity; prefetch depth = x bufs)
                xs = {}
                if t == 0 and W0 < CW:
                    t8 = pool.tile([128, F], i8, tag="t8", bufs=tbufs)
                    for j in range(NCC):
                        nvs[j] = pool.tile(
                            [128, CW], f32, tag=f"nv{j}", bufs=2, name=f"nv0_{j}"
                        )
                    x0bufs = int(os.environ.get("LIF_X0BUFS", "6"))
                    ld_split = os.environ.get("LIF_T0SPLIT", "0") == "1"
                    for k in range(F // W0):
                        xt = pool.tile([128, W0], f32, tag="x0", bufs=x0bufs)
                        ld = nc.scalar if (ld_split and k % 2) else nc.sync
                        ld.dma_start(
                            out=xt, in_=x[0][:, k * W0 : (k + 1) * W0]
                        )
                        sl = slice(k * W0, (k + 1) * W0)
                        nc.scalar.activation(t8[:, sl], xt, AF.Sign, bias=bias)
                        j = (k * W0) // CW
                        nsub = slice(k * W0 - j * CW, (k + 1) * W0 - j * CW)
                        nc.vector.scalar_tensor_tensor(
                            nvs[j][:, nsub], t8[:, sl], 0.0, xt, A.max, A.subtract
                        )
                    st_eng.dma_start(out=y[0], in_=t8)
                    continue
                for i in range(NCH):
                    xt = pool.tile([128, W], f32, tag="x")
                    ld = nc.scalar if (ringsplit and i % 2) else nc.sync
                    ld.dma_start(out=xt, in_=x[t][:, i * W : (i + 1) * W])
                    xs[i] = xt

                # one contiguous int8 spike tile per timestep -> 1 store
                t8 = pool.tile([128, F], i8, tag="t8", bufs=tbufs)

                presets = int(os.environ.get("LIF_PRESETS", "0"))

                def emit_reset(j, u):
                    # nv_t = max(t,0) - u   (= s - u = -v)
                    sl = slice(j * CW, (j + 1) * CW)
                    nv = pool.tile([128, CW], f32, tag=f"nv{j}", bufs=2)
                    if presets and j % (NCC // presets or 1) == 1:
                        # offload: Act computes relu(t) -> f32, gpsimd subs
                        rh = pool.tile(
                            [128, CW], f32, tag=f"rh{j}", bufs=2, name=f"rh_{j}"
                        )
                        nc.scalar.activation(rh, t8[:, sl], AF.Relu)
                        nc.gpsimd.tensor_sub(nv, rh, u)
                    else:
                        nc.vector.scalar_tensor_tensor(
                            nv, t8[:, sl], 0.0, u, A.max, A.subtract
                        )
                    nvs[j] = nv

                pending = None  # (j, u) whose reset is deferred one chunk
                for j in range(NCC):
                    sl = slice(j * CW, (j + 1) * CW)  # within t8 / F
                    xsl = xs[j // SUB][:, (j % SUB) * CW : (j % SUB + 1) * CW]
                    if t == T - 1 and j == NCC - 1:
                        # final chunk: split integrate/sign/store fine so the
                        # serial tail chain overlaps instead of serializing
                        u = pool.tile([128, CW], f32, tag="u", bufs=ubufs)
                        nq = 4
                        q = CW // nq
                        for k in range(nq):
                            usub = slice(k * q, (k + 1) * q)
                            ysub = slice(j * CW + k * q, j * CW + (k + 1) * q)
                            nc.vector.scalar_tensor_tensor(
                                u[:, usub],
                                nvs[j][:, usub],
                                -0.5,
                                xsl[:, usub],
                                A.mult,
                                A.add,
                            )
                            nc.scalar.activation(
                                t8[:, ysub], u[:, usub], AF.Sign, bias=bias
                            )
                            st_eng.dma_start(out=y[t][:, ysub], in_=t8[:, ysub])
                        continue
                    if t == 0:
                        u = xsl
                    else:
                        # u_t = nv_{t-1} * -0.5 + x_t
                        u = pool.tile([128, CW], f32, tag="u", bufs=ubufs)
                        nc.vector.scalar_tensor_tensor(
                            u, nvs[j], -0.5, xsl, A.mult, A.add
                        )
                    nc.scalar.activation(t8[:, sl], u, AF.Sign, bias=bias)
                    if t < T - 1:
                        if skew:
                            if pending is not None:
                                emit_reset(*pending)
                            pending = (j, u)
                        else:
                            emit_reset(j, u)
                        if chunkstore_all:
                            # per-chunk store: its sem wait blocks later
                            # sync-stream triggers (next t's loads) less than
                            # one big store waiting on all NCC signs would
                            st_eng.dma_start(out=y[t][:, sl], in_=t8[:, sl])
                    else:
                        # last timestep: store per compute-chunk so the tail
                        # store is small
                        st_eng.dma_start(out=y[t][:, sl], in_=t8[:, sl])
                if pending is not None:
                    emit_reset(*pending)

                if t < T - 1 and not chunkstore_all:
                    st_eng.dma_start(out=y[t], in_=t8)

    nc.compile()
    return nc


def _build_u8():
    """Fallback: previous all-DVE scheme with uint8 output (106us)."""
    import concourse.bacc as bacc
    import concourse.mybir as mybir
    from concourse import tile

    f32 = mybir.dt.float32
    u8 = mybir.dt.uint8
    mult = mybir.AluOpType.mult
    add = mybir.AluOpType.add
    is_ge = mybir.AluOpType.is_ge

    W = min(int(os.environ.get("LIF_W", "2048")), F)
    NCH = F // W
    assert F % W == 0

    nc = bacc.Bacc("TRN2", target_bir_lowering=False, debug=False, num_devices=NCORES)
    x = nc.dram_tensor("x", [T, 128, F], f32, kind="ExternalInput").ap()
    y = nc.dram_tensor("y", [T, 128, F], u8, kind="ExternalOutput").ap()

    xbufs = int(os.environ.get("LIF_XBUFS", "6"))
    with tile.TileContext(nc) as tc:
        with tc.tile_pool(name="p", bufs=xbufs) as pool:
            vs = {}
            for t in range(T):
                xs = {}
                for i in range(NCH):
                    xt = pool.tile([128, W], f32, tag="x")
                    nc.sync.dma_start(out=xt, in_=x[t][:, i * W : (i + 1) * W])
                    xs[i] = xt

                if t == 0:
                    us = xs
                else:
                    us = {}
                    for i in range(NCH):
                        u = pool.tile([128, W], f32, tag="u", bufs=4)
                        nc.vector.scalar_tensor_tensor(
                            u, vs[i], 0.5, xs[i], mult, add
                        )
                        us[i] = u

                ss = {}
                for i in range(NCH):
                    st = pool.tile([128, W], u8, tag="s", bufs=5)
                    nc.vector.tensor_scalar(st, us[i], 1.0, None, is_ge)
                    ss[i] = st
                if t < T - 1:
                    for i in range(NCH):
                        v = pool.tile([128, W], f32, tag=f"v{i}", bufs=2)
                        nc.vector.tensor_sub(v, us[i], ss[i])
                        vs[i] = v
                for i in range(NCH):
                    nc.scalar.dma_start(out=y[t][:, i * W : (i + 1) * W], in_=ss[i])

    nc.compile()
    return nc


def _get_nc():
    if "nc" not in _CACHE:
        scheme = os.environ.get("LIF_SCHEME", "sign")
        _CACHE["scheme"] = scheme
        _CACHE["nc"] = _build_u8() if scheme == "u8" else _build_sign()
    return _CACHE["nc"]


def kernel(x: np.ndarray) -> np.ndarray:
    global LAST_EXEC_NS, LAST_TRACE
    from concourse.bass_utils import run_bass_kernel_spmd

    x = np.ascontiguousarray(np.asarray(x), dtype=np.float32)
    assert x.shape == (T * B, C, 32, 32), x.shape
    xv = x.reshape(T, B, C, HW)

    in_maps = []
    for m in range(NCORES):
        shard = np.ascontiguousarray(xv[:, m * BLOC : (m + 1) * BLOC]).reshape(
            T, 128, F
        )
        in_maps.append({"x": shard})

    nc = _get_nc()
    trace = os.environ.get("LIF_TRACE") == "1"
    res = run_bass_kernel_spmd(nc, in_maps, core_ids=list(range(NCORES)), trace=trace)
    LAST_EXEC_NS = res.exec_time_ns
    if res.instructions_and_trace is not None:
        LAST_TRACE = res.instructions_and_trace[1]

    out = np.empty((T, B, C, HW), dtype=np.float32)
    for m in range(NCORES):
        raw = np.asarray(res.results[m]["y"])
        if _CACHE.get("scheme", "sign") == "u8":
            sp = raw.view(np.uint8)
        else:
            # int8 sign bytes {-1,0,1} -> spike iff == 1
            sp = raw.view(np.int8) == 1
        out[:, m * BLOC : (m + 1) * BLOC] = sp.astype(np.float32).reshape(
            T, BLOC, C, HW
        )
    return out.reshape(T * B, C, 32, 32)


# revision 21
# speedup vs baseline: 1.0212x; 1.0212x over previous
"""LIF spike-train kernel for Trainium2 (Bass/Tile), data-parallel over 8 cores.

Reference semantics (T=4, tau=0.5, thresh=1.0), per element:
    mem = 0
    for t in range(4):
        mem = mem*0.5 + x[t]
        s[t] = (mem - 1 >= 0)
        mem = mem - s[t]

x: [T*B, C, H, W] = [256, 128, 32, 32] f32, viewed as [4, 64, 128, 1024].
Batch dim (64) is sharded 8-ways; each core streams [4, 8, 128, 1024],
flattened to x,y: [T, 128, F] (F = 8192).

Scheme "SignFlow" (measured-engine-balance redesign; ~76us vs the 107.5us
all-DVE baseline):
  - Act (scalar) engine does all compares: t = Sign(u - c) -> int8 {-1,0,1},
    where c = nextafter(1,0). Exact: u >= 1  <=>  u - c > 0, and u == c
    (sign 0) is correctly "no spike". The int8 tile IS the output; the host
    maps byte==1 -> 1.0f. 1-byte output also cuts write traffic 4x.
    Act measured ~2000ns per [128,2048] op, ~32us total.
  - DVE does only the two 2-source ops per step (measured flat ~2290ns per
    [128,2048] op regardless of operand dtypes/spaces; 1-source ops 1216ns):
      reset:     nv_t = max(t_t, 0) - u_t            (= -(u - s), sign
                 absorbed into the next integrate)
      integrate: u_{t+1} = nv_t * -0.5 + x_{t+1}     (= 0.5*v + x)
    fp32 mult by 0.5 is exact, adds round identically to the reference.
    24 ops x 2290 = 54.8us busy, gapless = the critical engine.
  - The reset is emitted one chunk behind its sign (software-pipeline skew)
    so the DVE stream never stalls waiting for Act.
  - Pool/PE idle: gpsimd measured 5.4us (tt) to 37us (ts) per op and its
    SBUF-port contention slows the DVE (tested 90us); a PE/PSUM hybrid
    (Act copy u->psum + PE accum -0.5*I@s_fp8, validated exact on HW) nets
    only ~2us because PSUM fits half the working set and the fp8 compare
    costs 1472 vs 1218 on DVE -- not worth the complexity.
  - Remaining time = ~6.5us engine-wake preamble (bass const-AP memsets on
    slow-waking gpsimd gate the init barrier) + ~7us first-load ramp +
    54.8us DVE + ~5us tail (store + drain + semaphore-clear storm).

All spike decisions are bit-exact vs the fp32 reference (rel err 0.0).
"""

import os
import sys

sys.path.insert(0, "/opt/trn_rl_repo")

import numpy as np

T = 4
B = 64
C = 128
HW = 1024
NCORES = 8
BLOC = B // NCORES  # 8 batch elements per core
F = BLOC * C * HW // 128  # 8192 flat free width per t-block
C_THRESH = float(np.nextafter(np.float32(1.0), np.float32(0.0)))

LAST_EXEC_NS = None
LAST_TRACE = None

_CACHE = {}


def _build_sign():
    """SignFlow kernel: Act compares, DVE 2-src ops, int8 spike output."""
    import concourse.bacc as bacc
    import concourse.mybir as mybir
    from concourse import tile

    f32 = mybir.dt.float32
    i8 = mybir.dt.int8
    A = mybir.AluOpType
    AF = mybir.ActivationFunctionType

    W = min(int(os.environ.get("LIF_W", "2048")), F)  # load-chunk width
    CW = min(int(os.environ.get("LIF_CW", str(W))), W)  # compute-chunk width
    NCH = F // W
    SUB = W // CW
    NCC = F // CW
    assert F % W == 0 and W % CW == 0

    nc = bacc.Bacc("TRN2", target_bir_lowering=False, debug=False, num_devices=NCORES)
    x = nc.dram_tensor("x", [T, 128, F], f32, kind="ExternalInput").ap()
    y = nc.dram_tensor("y", [T, 128, F], i8, kind="ExternalOutput").ap()

    xbufs = int(os.environ.get("LIF_XBUFS", "6"))
    ubufs = int(os.environ.get("LIF_UBUFS", "6"))
    tbufs = int(os.environ.get("LIF_TBUFS", "2"))
    ringsplit = os.environ.get("LIF_RINGSPLIT") == "1"
    st_eng_name = os.environ.get("LIF_STORE_ENG", "sync")
    skew = os.environ.get("LIF_SKEW", "1") == "1"
    chunkstore_all = os.environ.get("LIF_CHUNKSTORE_ALL", "0") == "1"

    with tile.TileContext(nc) as tc:
        with tc.tile_pool(name="p", bufs=xbufs) as pool:
            bias = pool.tile([128, 1], f32, tag="bias", bufs=1)
            warm = pool.tile([128, 1], i8, tag="warm", bufs=1)
            # memset on DVE (not gpsimd): gpsimd's slow wake already gates the
            # init barrier via the bass const-AP memsets; don't add to it.
            nc.vector.memset(bias, -C_THRESH)
            # dummy Sign to pull the ACT table load into the preamble
            nc.scalar.activation(warm, bias, AF.Sign, bias=bias)
            st_eng = nc.scalar if st_eng_name == "scalar" else nc.sync

            # t=0 runs at fine granularity (W0) so the first compute starts
            # as soon as a small first load lands; resets write into
            # CW-wide nv tiles so t=1.. runs at coarse granularity.
            W0 = min(int(os.environ.get("LIF_W0", str(CW))), CW)
            nvs = {}
            for t in range(T):
                # loads (load-chunk granularity; prefetch depth = x bufs)
                xs = {}
                if t == 0 and W0 < CW:
                    t8 = pool.tile([128, F], i8, tag="t8", bufs=tbufs)
                    for j in range(NCC):
                        nvs[j] = pool.tile(
                            [128, CW], f32, tag=f"nv{j}", bufs=2, name=f"nv0_{j}"
                        )
                    x0bufs = int(os.environ.get("LIF_X0BUFS", "6"))
                    ld_split = os.environ.get("LIF_T0SPLIT", "0") == "1"
                    for k in range(F // W0):
                        xt = pool.tile([128, W0], f32, tag="x0", bufs=x0bufs)
                        ld = nc.scalar if (ld_split and k % 2) else nc.sync
                        ld.dma_start(
                            out=xt, in_=x[0][:, k * W0 : (k + 1) * W0]
                        )
                        sl = slice(k * W0, (k + 1) * W0)
                        nc.scalar.activation(t8[:, sl], xt, AF.Sign, bias=bias)
                        j = (k * W0) // CW
                        nsub = slice(k * W0 - j * CW, (k + 1) * W0 - j * CW)
                        nc.vector.scalar_tensor_tensor(
                            nvs[j][:, nsub], t8[:, sl], 0.0, xt, A.max, A.subtract
                        )
                    st_eng.dma_start(out=y[0], in_=t8)
                    continue
                faststart = (
                    t == 0
                    and SUB == 1
                    and CW >= 1024
                    and os.environ.get("LIF_FASTSTART", "1") == "1"
                )
                for i in range(NCH):
                    if faststart and i == 0:
                        continue  # chunk 0 of t0 arrives via the fine sub-loads
                    xt = pool.tile([128, W], f32, tag="x")
                    ld = nc.scalar if (ringsplit and i % 2) else nc.sync
                    ld.dma_start(out=xt, in_=x[t][:, i * W : (i + 1) * W])
                    xs[i] = xt

                # one contiguous int8 spike tile per timestep -> 1 store
                t8 = pool.tile([128, F], i8, tag="t8", bufs=tbufs)

                presets = int(os.environ.get("LIF_PRESETS", "0"))

                def emit_reset(j, u):
                    # nv_t = max(t,0) - u   (= s - u = -v)
                    sl = slice(j * CW, (j + 1) * CW)
                    nv = pool.tile([128, CW], f32, tag=f"nv{j}", bufs=2)
                    if presets and j % (NCC // presets or 1) == 1:
                        # offload: Act computes relu(t) -> f32, gpsimd subs
                        rh = pool.tile(
                            [128, CW], f32, tag=f"rh{j}", bufs=2, name=f"rh_{j}"
                        )
                        nc.scalar.activation(rh, t8[:, sl], AF.Relu)
                        nc.gpsimd.tensor_sub(nv, rh, u)
                    else:
                        nc.vector.scalar_tensor_tensor(
                            nv, t8[:, sl], 0.0, u, A.max, A.subtract
                        )
                    nvs[j] = nv

                pending = None  # (j, u) whose reset is deferred one chunk
                j_start = 0
                if faststart:
                    # first chunk of t0 arrives as [512,512,1024,...] sub-loads
                    # so the first sign/reset fire ~4us earlier; everything
                    # downstream (the gapless DVE stream) shifts left with it
                    nv = pool.tile([128, CW], f32, tag="nv0", bufs=2, name="nvf")
                    widths = [512, 512] + [1024] * ((CW - 1024) // 1024)
                    off = 0
                    for wsub in widths:
                        xf = pool.tile(
                            [128, wsub], f32, tag=f"xf{off}", bufs=1, name=f"xf{off}"
                        )
                        nc.sync.dma_start(out=xf, in_=x[0][:, off : off + wsub])
                        nc.scalar.activation(
                            t8[:, off : off + wsub], xf, AF.Sign, bias=bias
                        )
                        nc.vector.scalar_tensor_tensor(
                            nv[:, off : off + wsub],
                            t8[:, off : off + wsub],
                            0.0,
                            xf,
                            A.max,
                            A.subtract,
                        )
                        off += wsub
                    nvs[0] = nv
                    j_start = 1
                for j in range(j_start, NCC):
                    sl = slice(j * CW, (j + 1) * CW)  # within t8 / F
                    xsl = xs[j // SUB][:, (j % SUB) * CW : (j % SUB + 1) * CW]
                    if t == T - 1 and j == NCC - 1:
                        # final chunk: split integrate/sign/store fine so the
                        # serial tail chain overlaps instead of serializing
                        u = pool.tile([128, CW], f32, tag="u", bufs=ubufs)
                        nq = 4
                        q = CW // nq
                        for k in range(nq):
                            usub = slice(k * q, (k + 1) * q)
                            ysub = slice(j * CW + k * q, j * CW + (k + 1) * q)
                            nc.vector.scalar_tensor_tensor(
                                u[:, usub],
                                nvs[j][:, usub],
                                -0.5,
                                xsl[:, usub],
                                A.mult,
                                A.add,
                            )
                            nc.scalar.activation(
                                t8[:, ysub], u[:, usub], AF.Sign, bias=bias
                            )
                            st_eng.dma_start(out=y[t][:, ysub], in_=t8[:, ysub])
                        continue
                    if t == 0:
                        u = xsl
                    else:
                        # u_t = nv_{t-1} * -0.5 + x_t
                        u = pool.tile([128, CW], f32, tag="u", bufs=ubufs)
                        nc.vector.scalar_tensor_tensor(
                            u, nvs[j], -0.5, xsl, A.mult, A.add
                        )
                    nc.scalar.activation(t8[:, sl], u, AF.Sign, bias=bias)
                    if t < T - 1:
                        if skew:
                            if pending is not None:
                                emit_reset(*pending)
                            pending = (j, u)
                        else:
                            emit_reset(j, u)
                        if chunkstore_all:
                            # per-chunk store: its sem wait blocks later
                            # sync-stream triggers (next t's loads) less than
                            # one big store waiting on all NCC signs would
                            st_eng.dma_start(out=y[t][:, sl], in_=t8[:, sl])
                    else:
                        # last timestep: store per compute-chunk so the tail
                        # store is small
                        st_eng.dma_start(out=y[t][:, sl], in_=t8[:, sl])
                if pending is not None:
                    emit_reset(*pending)

                if t < T - 1 and not chunkstore_all:
                    st_eng.dma_start(out=y[t], in_=t8)

    nc.compile()
    return nc


def _build_u8():
    """Fallback: previous all-DVE scheme with uint8 output (106us)."""
    import concourse.bacc as bacc
    import concourse.mybir as mybir
    from concourse import tile

    f32 = mybir.dt.float32
    u8 = mybir.dt.uint8
    mult = mybir.AluOpType.mult
    add = mybir.AluOpType.add
    is_ge = mybir.AluOpType.is_ge

    W = min(int(os.environ.get("LIF_W", "2048")), F)
    NCH = F // W
    assert F % W == 0

    nc = bacc.Bacc("TRN2", target_bir_lowering=False, debug=False, num_devices=NCORES)
    x = nc.dram_tensor("x", [T, 128, F], f32, kind="ExternalInput").ap()
    y = nc.dram_tensor("y", [T, 128, F], u8, kind="ExternalOutput").ap()

    xbufs = int(os.environ.get("LIF_XBUFS", "6"))
    with tile.TileContext(nc) as tc:
        with tc.tile_pool(name="p", bufs=xbufs) as pool:
            vs = {}
            for t in range(T):
                xs = {}
                for i in range(NCH):
                    xt = pool.tile([128, W], f32, tag="x")
                    nc.sync.dma_start(out=xt, in_=x[t][:, i * W : (i + 1) * W])
                    xs[i] = xt

                if t == 0:
                    us = xs
                else:
                    us = {}
                    for i in range(NCH):
                        u = pool.tile([128, W], f32, tag="u", bufs=4)
                        nc.vector.scalar_tensor_tensor(
                            u, vs[i], 0.5, xs[i], mult, add
                        )
                        us[i] = u

                ss = {}
                for i in range(NCH):
                    st = pool.tile([128, W], u8, tag="s", bufs=5)
                    nc.vector.tensor_scalar(st, us[i], 1.0, None, is_ge)
                    ss[i] = st
                if t < T - 1:
                    for i in range(NCH):
                        v = pool.tile([128, W], f32, tag=f"v{i}", bufs=2)
                        nc.vector.tensor_sub(v, us[i], ss[i])
                        vs[i] = v
                for i in range(NCH):
                    nc.scalar.dma_start(out=y[t][:, i * W : (i + 1) * W], in_=ss[i])

    nc.compile()
    return nc


def _get_nc():
    if "nc" not in _CACHE:
        scheme = os.environ.get("LIF_SCHEME", "sign")
        _CACHE["scheme"] = scheme
        _CACHE["nc"] = _build_u8() if scheme == "u8" else _build_sign()
    return _CACHE["nc"]


def kernel(x: np.ndarray) -> np.ndarray:
    global LAST_EXEC_NS, LAST_TRACE
    from concourse.bass_utils import run_bass_kernel_spmd

    x = np.ascontiguousarray(np.asarray(x), dtype=np.float32)
    assert x.shape == (T * B, C, 32, 32), x.shape
    xv = x.reshape(T, B, C, HW)

    in_maps = []
    for m in range(NCORES):
        shard = np.ascontiguousarray(xv[:, m * BLOC : (m + 1) * BLOC]).reshape(
            T, 128, F
        )
        in_maps.append({"x": shard})

    nc = _get_nc()
    trace = os.environ.get("LIF_TRACE") == "1"
    res = run_bass_kernel_spmd(nc, in_maps, core_ids=list(range(NCORES)), trace=trace)
    LAST_EXEC_NS = res.exec_time_ns
    if res.instructions_and_trace is not None:
        LAST_TRACE = res.instructions_and_trace[1]

    out = np.empty((T, B, C, HW), dtype=np.float32)
    for m in range(NCORES):
        raw = np.asarray(res.results[m]["y"])
        if _CACHE.get("scheme", "sign") == "u8":
            sp = raw.view(np.uint8)
        else:
            # int8 sign bytes {-1,0,1} -> spike iff == 1
            sp = raw.view(np.int8) == 1
        out[:, m * BLOC : (m + 1) * BLOC] = sp.astype(np.float32).reshape(
            T, BLOC, C, HW
        )
    return out.reshape(T * B, C, 32, 32)


# revision 36
# speedup vs baseline: 1.0581x; 1.0361x over previous
"""LIF spike-train kernel for Trainium2 (Bass/Tile), data-parallel over 8 cores.

Reference semantics (T=4, tau=0.5, thresh=1.0), per element:
    mem = 0
    for t in range(4):
        mem = mem*0.5 + x[t]
        s[t] = (mem - 1 >= 0)
        mem = mem - s[t]

x: [T*B, C, H, W] = [256, 128, 32, 32] f32, viewed as [4, 64, 128, 1024].
Batch dim (64) is sharded 8-ways; each core streams [4, 8, 128, 1024],
flattened to x,y: [T, 128, F] (F = 8192).

Scheme "SignFlow" (measured-engine-balance redesign; ~76us vs the 107.5us
all-DVE baseline):
  - Act (scalar) engine does all compares: t = Sign(u - c) -> int8 {-1,0,1},
    where c = nextafter(1,0). Exact: u >= 1  <=>  u - c > 0, and u == c
    (sign 0) is correctly "no spike". The int8 tile IS the output; the host
    maps byte==1 -> 1.0f. 1-byte output also cuts write traffic 4x.
    Act measured ~2000ns per [128,2048] op, ~32us total.
  - DVE does only the two 2-source ops per step (measured flat ~2290ns per
    [128,2048] op regardless of operand dtypes/spaces; 1-source ops 1216ns):
      reset:     nv_t = max(t_t, 0) - u_t            (= -(u - s), sign
                 absorbed into the next integrate)
      integrate: u_{t+1} = nv_t * -0.5 + x_{t+1}     (= 0.5*v + x)
    fp32 mult by 0.5 is exact, adds round identically to the reference.
    24 ops x 2290 = 54.8us busy, gapless = the critical engine.
  - The reset is emitted one chunk behind its sign (software-pipeline skew)
    so the DVE stream never stalls waiting for Act.
  - Pool/PE idle: gpsimd measured 5.4us (tt) to 37us (ts) per op and its
    SBUF-port contention slows the DVE (tested 90us). A PE/PSUM hybrid
    (Act copy 0.5u->psum bank + PE accum -0.5*I@s_fp8, LIF_PE env) ran
    ~74.2us but is numerically WRONG under pipelining: single-shot the
    pattern is exact (micro-validated), but repeated matmul(start=False)
    accumulation onto Act-initialized banks corrupts P-chunks at t>=1
    even with pool-rotated banksets -- left disabled (LIF_PE=0).
  - Remaining time = ~6.5us engine-wake preamble (bass const-AP memsets on
    slow-waking gpsimd gate the init barrier) + ~7us first-load ramp +
    54.8us DVE + ~5us tail (store + drain + semaphore-clear storm).

All spike decisions are bit-exact vs the fp32 reference (rel err 0.0).
"""

import os
import sys

sys.path.insert(0, "/opt/trn_rl_repo")

import numpy as np

T = 4
B = 64
C = 128
HW = 1024
NCORES = 8
BLOC = B // NCORES  # 8 batch elements per core
F = BLOC * C * HW // 128  # 8192 flat free width per t-block
C_THRESH = float(np.nextafter(np.float32(1.0), np.float32(0.0)))

LAST_EXEC_NS = None
LAST_TRACE = None

_CACHE = {}


def _build_sign():
    """SignFlow kernel: Act compares, DVE 2-src ops, int8 spike output."""
    import concourse.bacc as bacc
    import concourse.mybir as mybir
    from concourse import tile

    f32 = mybir.dt.float32
    i8 = mybir.dt.int8
    fp8 = mybir.dt.float8e4
    bf16 = mybir.dt.bfloat16
    A = mybir.AluOpType
    AF = mybir.ActivationFunctionType

    W = min(int(os.environ.get("LIF_W", "2048")), F)  # load-chunk width
    CW = min(int(os.environ.get("LIF_CW", str(W))), W)  # compute-chunk width
    NCH = F // W
    SUB = W // CW
    NCC = F // CW
    assert F % W == 0 and W % CW == 0

    # npe chunks run the PE/PSUM scheme: P = 0.5u - 0.5s is formed in PSUM by
    # an Act copy (0.5u) + a PE accumulate (-0.5*I @ s_fp8), replacing the
    # DVE reset (2291ns) with an fp8 compare (1472ns) on the critical steps.
    # With the PE scheme the spike bytes are fp8e4 (1.0 = 0x38); Act Sign
    # then also writes fp8 (-1 -> 0xB8), host decodes spike := byte == 0x38.
    npe = int(os.environ.get("LIF_PE", "0"))
    # PSUM: 8 banks total; each P-chunk needs a double-buffered bankset
    # (fresh pool generation per cycle so the tile framework emits proper
    # cross-engine sems for the Act-write -> PE-accum -> DVE-read chain)
    npe = min(npe, NCC, 8 // (CW * 4 // 2048 * 2))

    nc = bacc.Bacc("TRN2", target_bir_lowering=False, debug=False, num_devices=NCORES)
    x = nc.dram_tensor("x", [T, 128, F], f32, kind="ExternalInput").ap()
    sdt = fp8 if npe else i8
    y = nc.dram_tensor("y", [T, 128, F], sdt, kind="ExternalOutput").ap()
    if npe:
        wmat = nc.dram_tensor("w", [128, 128], bf16, kind="ExternalInput").ap()

    xbufs = int(os.environ.get("LIF_XBUFS", "6"))
    ubufs = int(os.environ.get("LIF_UBUFS", "6"))
    tbufs = int(os.environ.get("LIF_TBUFS", "2"))
    ringsplit = os.environ.get("LIF_RINGSPLIT") == "1"
    st_eng_name = os.environ.get("LIF_STORE_ENG", "sync")
    skew = os.environ.get("LIF_SKEW", "1") == "1"
    chunkstore_all = os.environ.get("LIF_CHUNKSTORE_ALL", "0") == "1"

    with tile.TileContext(nc) as tc:
        with tc.tile_pool(name="p", bufs=xbufs) as pool, tc.psum_pool(
            name="ps", bufs=1
        ) as ppool:
            Bs = {}
            if npe:
                wt = pool.tile([128, 128], bf16, tag="wt", bufs=1)
                nc.sync.dma_start(out=wt, in_=wmat)
            bias = pool.tile([128, 1], f32, tag="bias", bufs=1)
            warm = pool.tile([128, 1], i8, tag="warm", bufs=1)
            # memset on DVE (not gpsimd): gpsimd's slow wake already gates the
            # init barrier via the bass const-AP memsets; don't add to it.
            nc.vector.memset(bias, -C_THRESH)
            # dummy Sign to pull the ACT table load into the preamble
            nc.scalar.activation(warm, bias, AF.Sign, bias=bias)
            st_eng = nc.scalar if st_eng_name == "scalar" else nc.sync

            # t=0 runs at fine granularity (W0) so the first compute starts
            # as soon as a small first load lands; resets write into
            # CW-wide nv tiles so t=1.. runs at coarse granularity.
            W0 = min(int(os.environ.get("LIF_W0", str(CW))), CW)
            defer_store = os.environ.get("LIF_DEFER_STORE", "1") == "1"
            nvs = {}
            store_pending = None  # (t, t8) big store deferred past next loads
            for t in range(T):
                # loads (load-chunk granularity; prefetch depth = x bufs)
                xs = {}
                if t == 0 and W0 < CW:
                    t8 = pool.tile([128, F], i8, tag="t8", bufs=tbufs)
                    for j in range(NCC):
                        nvs[j] = pool.tile(
                            [128, CW], f32, tag=f"nv{j}", bufs=2, name=f"nv0_{j}"
                        )
                    x0bufs = int(os.environ.get("LIF_X0BUFS", "6"))
                    ld_split = os.environ.get("LIF_T0SPLIT", "0") == "1"
                    for k in range(F // W0):
                        xt = pool.tile([128, W0], f32, tag="x0", bufs=x0bufs)
                        ld = nc.scalar if (ld_split and k % 2) else nc.sync
                        ld.dma_start(
                            out=xt, in_=x[0][:, k * W0 : (k + 1) * W0]
                        )
                        sl = slice(k * W0, (k + 1) * W0)
                        nc.scalar.activation(t8[:, sl], xt, AF.Sign, bias=bias)
                        j = (k * W0) // CW
                        nsub = slice(k * W0 - j * CW, (k + 1) * W0 - j * CW)
                        nc.vector.scalar_tensor_tensor(
                            nvs[j][:, nsub], t8[:, sl], 0.0, xt, A.max, A.subtract
                        )
                    st_eng.dma_start(out=y[0], in_=t8)
                    continue
                faststart = (
                    t == 0
                    and SUB == 1
                    and CW >= 1024
                    and os.environ.get("LIF_FASTSTART", "0") == "1"
                )
                for i in range(NCH):
                    if faststart and i == 0:
                        continue  # chunk 0 of t0 arrives via the fine sub-loads
                    xt = pool.tile([128, W], f32, tag="x")
                    ld = nc.scalar if (ringsplit and i % 2) else nc.sync
                    ld.dma_start(out=xt, in_=x[t][:, i * W : (i + 1) * W])
                    xs[i] = xt

                if store_pending is not None:
                    # emit the previous timestep's store AFTER this t's load
                    # triggers: its sem wait (all signs of t-1) would otherwise
                    # block these loads in the in-order sync stream
                    pt, pt8 = store_pending
                    st_eng.dma_start(out=y[pt], in_=pt8)
                    store_pending = None

                # one contiguous 1-byte spike tile per timestep -> 1 store
                t8 = pool.tile([128, F], sdt, tag="t8", bufs=tbufs)

                presets = int(os.environ.get("LIF_PRESETS", "0"))

                def emit_reset(j, u):
                    # nv_t = max(t,0) - u   (= s - u = -v)
                    sl = slice(j * CW, (j + 1) * CW)
                    nv = pool.tile([128, CW], f32, tag=f"nv{j}", bufs=2)
                    if presets and j % (NCC // presets or 1) == 1:
                        # offload: Act computes relu(t) -> f32, gpsimd subs
                        rh = pool.tile(
                            [128, CW], f32, tag=f"rh{j}", bufs=2, name=f"rh_{j}"
                        )
                        nc.scalar.activation(rh, t8[:, sl], AF.Relu)
                        nc.gpsimd.tensor_sub(nv, rh, u)
                    else:
                        nc.vector.scalar_tensor_tensor(
                            nv, t8[:, sl], 0.0, u, A.max, A.subtract
                        )
                    nvs[j] = nv

                pending = None  # (j, u) whose reset is deferred one chunk
                j_start = 0
                if faststart:
                    # first chunk of t0 arrives as [512,512,1024,...] sub-loads
                    # so the first sign/reset fire ~4us earlier; everything
                    # downstream (the gapless DVE stream) shifts left with it
                    nv = pool.tile([128, CW], f32, tag="nv0", bufs=2, name="nvf")
                    widths = [512, 512] + [1024] * ((CW - 1024) // 1024)
                    off = 0
                    for wsub in widths:
                        xf = pool.tile(
                            [128, wsub], f32, tag=f"xf{off}", bufs=1, name=f"xf{off}"
                        )
                        nc.sync.dma_start(out=xf, in_=x[0][:, off : off + wsub])
                        nc.scalar.activation(
                            t8[:, off : off + wsub], xf, AF.Sign, bias=bias
                        )
                        nc.vector.scalar_tensor_tensor(
                            nv[:, off : off + wsub],
                            t8[:, off : off + wsub],
                            0.0,
                            xf,
                            A.max,
                            A.subtract,
                        )
                        off += wsub
                    nvs[0] = nv
                    j_start = 1
                for j in range(j_start, NCC):
                    sl = slice(j * CW, (j + 1) * CW)  # within t8 / F
                    xsl = xs[j // SUB][:, (j % SUB) * CW : (j % SUB + 1) * CW]
                    if j < npe:
                        # PE/PSUM chunk: u = x + P (P in psum), fp8 compare is
                        # the output byte AND the PE matmul input; Act + PE
                        # rebuild P = 0.5u - 0.5s in the same psum bankset.
                        if t == 0:
                            u = xsl
                        else:
                            u = pool.tile([128, CW], f32, tag="u", bufs=ubufs)
                            nc.vector.scalar_tensor_tensor(
                                u, xsl, 0.0, Bs[j], A.add, A.add
                            )
                        # s = (u >= 1) -> fp8 {0,1} written into the t8 slice
                        nc.vector.tensor_scalar(t8[:, sl], u, 1.0, None, A.is_ge)
                        if t < T - 1:
                            # fresh psum generation per cycle (pool rotation
                            # emits the cross-engine sems)
                            Bn = ppool.tile(
                                [128, CW], f32, tag=f"B{j}", bufs=2, name=f"B_{j}"
                            )
                            # B = 0.5*u
                            nc.scalar.activation(Bn, u, AF.Copy, 0.0, 0.5)
                            # B += (-0.5 I) @ s, one matmul per psum bank
                            for k in range(CW // 512):
                                bsl = slice(k * 512, (k + 1) * 512)
                                tsl = slice(j * CW + k * 512, j * CW + (k + 1) * 512)
                                nc.tensor.matmul(
                                    Bn[:, bsl],
                                    wt,
                                    t8[:, tsl],
                                    start=False,
                                    stop=True,
                                    skip_group_check=True,
                                )
                            Bs[j] = Bn
                        continue
                    if t == T - 1 and j == NCC - 1:
                        # final chunk: split integrate/sign/store fine so the
                        # serial tail chain overlaps instead of serializing
                        u = pool.tile([128, CW], f32, tag="u", bufs=ubufs)
                        nq = 4
                        q = CW // nq
                        for k in range(nq):
                            usub = slice(k * q, (k + 1) * q)
                            ysub = slice(j * CW + k * q, j * CW + (k + 1) * q)
                            nc.vector.scalar_tensor_tensor(
                                u[:, usub],
                                nvs[j][:, usub],
                                -0.5,
                                xsl[:, usub],
                                A.mult,
                                A.add,
                            )
                            nc.scalar.activation(
                                t8[:, ysub], u[:, usub], AF.Sign, bias=bias
                            )
                            st_eng.dma_start(out=y[t][:, ysub], in_=t8[:, ysub])
                        continue
                    if t == 0:
                        u = xsl
                    else:
                        # u_t = nv_{t-1} * -0.5 + x_t
                        u = pool.tile([128, CW], f32, tag="u", bufs=ubufs)
                        nc.vector.scalar_tensor_tensor(
                            u, nvs[j], -0.5, xsl, A.mult, A.add
                        )
                    nc.scalar.activation(t8[:, sl], u, AF.Sign, bias=bias)
                    if t < T - 1:
                        if skew:
                            if pending is not None:
                                emit_reset(*pending)
                            pending = (j, u)
                        else:
                            emit_reset(j, u)
                        if chunkstore_all:
                            # per-chunk store: its sem wait blocks later
                            # sync-stream triggers (next t's loads) less than
                            # one big store waiting on all NCC signs would
                            st_eng.dma_start(out=y[t][:, sl], in_=t8[:, sl])
                    else:
                        # last timestep: store per compute-chunk so the tail
                        # store is small
                        st_eng.dma_start(out=y[t][:, sl], in_=t8[:, sl])
                if pending is not None:
                    emit_reset(*pending)

                if t < T - 1 and not chunkstore_all:
                    if defer_store:
                        store_pending = (t, t8)
                    else:
                        st_eng.dma_start(out=y[t], in_=t8)
            if store_pending is not None:
                pt, pt8 = store_pending
                st_eng.dma_start(out=y[pt], in_=pt8)

    nc.compile()
    return nc


def _build_u8():
    """Fallback: previous all-DVE scheme with uint8 output (106us)."""
    import concourse.bacc as bacc
    import concourse.mybir as mybir
    from concourse import tile

    f32 = mybir.dt.float32
    u8 = mybir.dt.uint8
    mult = mybir.AluOpType.mult
    add = mybir.AluOpType.add
    is_ge = mybir.AluOpType.is_ge

    W = min(int(os.environ.get("LIF_W", "2048")), F)
    NCH = F // W
    assert F % W == 0

    nc = bacc.Bacc("TRN2", target_bir_lowering=False, debug=False, num_devices=NCORES)
    x = nc.dram_tensor("x", [T, 128, F], f32, kind="ExternalInput").ap()
    y = nc.dram_tensor("y", [T, 128, F], u8, kind="ExternalOutput").ap()

    xbufs = int(os.environ.get("LIF_XBUFS", "6"))
    with tile.TileContext(nc) as tc:
        with tc.tile_pool(name="p", bufs=xbufs) as pool:
            vs = {}
            for t in range(T):
                xs = {}
                for i in range(NCH):
                    xt = pool.tile([128, W], f32, tag="x")
                    nc.sync.dma_start(out=xt, in_=x[t][:, i * W : (i + 1) * W])
                    xs[i] = xt

                if t == 0:
                    us = xs
                else:
                    us = {}
                    for i in range(NCH):
                        u = pool.tile([128, W], f32, tag="u", bufs=4)
                        nc.vector.scalar_tensor_tensor(
                            u, vs[i], 0.5, xs[i], mult, add
                        )
                        us[i] = u

                ss = {}
                for i in range(NCH):
                    st = pool.tile([128, W], u8, tag="s", bufs=5)
                    nc.vector.tensor_scalar(st, us[i], 1.0, None, is_ge)
                    ss[i] = st
                if t < T - 1:
                    for i in range(NCH):
                        v = pool.tile([128, W], f32, tag=f"v{i}", bufs=2)
                        nc.vector.tensor_sub(v, us[i], ss[i])
                        vs[i] = v
                for i in range(NCH):
                    nc.scalar.dma_start(out=y[t][:, i * W : (i + 1) * W], in_=ss[i])

    nc.compile()
    return nc


def _get_nc():
    if "nc" not in _CACHE:
        scheme = os.environ.get("LIF_SCHEME", "sign")
        _CACHE["scheme"] = scheme
        _CACHE["nc"] = _build_u8() if scheme == "u8" else _build_sign()
    return _CACHE["nc"]


def kernel(x: np.ndarray) -> np.ndarray:
    global LAST_EXEC_NS, LAST_TRACE
    from concourse.bass_utils import run_bass_kernel_spmd

    x = np.ascontiguousarray(np.asarray(x), dtype=np.float32)
    assert x.shape == (T * B, C, 32, 32), x.shape
    xv = x.reshape(T, B, C, HW)

    npe = int(os.environ.get("LIF_PE", "0"))
    wI = None
    if npe:
        import ml_dtypes

        wI = (np.eye(128, dtype=np.float32) * -0.5).astype(ml_dtypes.bfloat16)

    in_maps = []
    for m in range(NCORES):
        shard = np.ascontiguousarray(xv[:, m * BLOC : (m + 1) * BLOC]).reshape(
            T, 128, F
        )
        im = {"x": shard}
        if wI is not None:
            im["w"] = wI
        in_maps.append(im)

    nc = _get_nc()
    trace = os.environ.get("LIF_TRACE") == "1"
    res = run_bass_kernel_spmd(nc, in_maps, core_ids=list(range(NCORES)), trace=trace)
    LAST_EXEC_NS = res.exec_time_ns
    if res.instructions_and_trace is not None:
        LAST_TRACE = res.instructions_and_trace[1]

    out = np.empty((T, B, C, HW), dtype=np.float32)
    for m in range(NCORES):
        raw = np.asarray(res.results[m]["y"])
        if _CACHE.get("scheme", "sign") == "u8":
            sp = raw.view(np.uint8)
        elif npe:
            # fp8e4 bytes: 1.0 = 0x38 (spike); 0x00 / 0xB8 (-1.0) = no spike
            sp = raw.view(np.uint8) == 0x38
        else:
            # int8 sign bytes {-1,0,1} -> spike iff == 1
            sp = raw.view(np.int8) == 1
        out[:, m * BLOC : (m + 1) * BLOC] = sp.astype(np.float32).reshape(
            T, BLOC, C, HW
        )
    return out.reshape(T * B, C, 32, 32)
